# revision 43
# baseline (speedup 1.0000x reference)
"""Trainium2 Bass kernel for nn_Encoder_74182675137046.

Reference computation (per image of 1024x1024 complex pixels):
    feats = [norm_row, norm_col, x0, x1]  per pixel     [N, 4]
    h   = relu((feats @ W1 + b1) @ W2 + b2)             [N, 128]
    out = h @ W3 + b3                                   [N, 128]
    result = (w * out).sum(0) / w.sum()                 [128]
with w = (x0 != 0), and norm_row/col normalized by masked min/max.

Algebraic folding (exact):
    fc1+fc2 fold:  h_pre = feats @ W12 + b12,  W12 = W1@W2, b12 = b1@W2 + b2
    pool/fc3 swap: (w*out).sum = (sum_p w_p*relu(h_pre_p)) @ W3 + w.sum()*b3
So the device only computes S = sum_p relu(h_pre_p)  (a [128] vector per
core); the tiny [128]x[128,128] tail runs on host in float64.

Device design (per core, 128 image rows = 131072 points):
  - rhs slots: one fresh [128, 2048] slot per fill (16 fills x 8192 pts).
    Group g occupies partitions 32g+k: k=0 x0, k=1 x1 (DMA'd per fill),
    k=2 norm_col, k=3 ones (const rows, DMA'd per fill -- small pieces so
    the few DMA engines serving our sparse partitions never serialize a
    big transfer in front of compute).
  - lhsT table: [128, 32*128]; per (group, slot) block rows are
    {v2, v3, v1, btab[:, image_row]} -- bias row folds b12 + nr*v0.
    DMA'd compactly in 2 column pieces on the SWDGE path.
  - 64 quarters of 2048 points; quarter Q: 4 matmuls (one per tile_position
    group, K=4, N=512, fp32r) into the 8-bank PSUM ring at banks 4Q..4Q+3
    (mod 8).
  - consumers walk the ring in a 16-bank periodic pattern
    [(A,3),(V,3),(A,2),(V,3),(A,3),(V,2)] (A=ScalarE relu+accum,
    V=VectorE max/add+accum), amortizing per-op overhead at 3-bank op size
    while leaving 2 free banks of refill margin.
  - Sync-wait discipline (walrus allows ONE sem wait per instruction):
    ldweights "observers" make the PE sequencer pre-observe DMA lanes and
    ACT ticks; tiny per-op "carriers" on ACT/DVE pre-observe their own
    engine's accumulator ticks. Consumers write junk SBUF buffers (never
    PSUM in place) so every engine instruction carries at most one wait.
"""

import numpy as np

import concourse.bass as bass
import concourse.tile as tile
from concourse import mybir
from concourse.bass_utils import run_bass_kernel_spmd
from concourse.tile_rust import add_dep_helper

H = 1024
W = 1024
D = 128
N_CORES = 8
ROWS_PER_CORE = H // N_CORES          # 128
NPTS = ROWS_PER_CORE * W              # 131072
CHUNK = 2 * W                         # 2048 pts per (group, fill) = 2 image rows
NGROUPS = 4
FILL_PTS = NGROUPS * CHUNK            # 8192
NFILLS = NPTS // FILL_PTS             # 16
NT = 512
NQ = 4 * NFILLS                       # 64 quarters
NBANKS = 4 * NQ                       # 256 bank-writes over the 8-bank ring
ROW_SLOTS = 2 * NFILLS                # lhsT blocks per group (32)

# Static bank ownership (PSUM same-bank access by both engines serializes,
# so each ring bank is always consumed by the same engine): ACT owns banks
# {0,1,2} of even quarters and {4} of odd; DVE owns {3} and {5,6,7}. The
# 3-bank ops amortize per-op overhead; per-engine work stays balanced.

F32 = mybir.dt.float32
F32R = mybir.dt.float32r
BF16 = mybir.dt.bfloat16

TRACE = False
LAST_RESULT = None

_NC_CACHE = None


def _ring_ops():
    ops = []
    for q in range(NQ):
        if q % 2 == 0:
            ops.append(("A", 4 * q, 3))
            ops.append(("V", 4 * q + 3, 1))
        else:
            ops.append(("A", 4 * q, 1))
            ops.append(("V", 4 * q + 1, 3))
    return ops


def _build_bass():
    """Build the SPMD Bass program (same program on all 8 cores)."""
    global _NC_CACHE
    if _NC_CACHE is not None:
        return _NC_CACHE

    nc = bass.Bass()

    xd = nc.dram_tensor("xd", [NFILLS, 2, NGROUPS, CHUNK], F32R,
                        kind="ExternalInput")
    lwt4 = nc.dram_tensor("lwt4", [4, NGROUPS, 128 * ROW_SLOTS], F32R,
                          kind="ExternalInput")
    ncpre = nc.dram_tensor("ncpre", [NGROUPS, CHUNK], F32R,
                           kind="ExternalInput")
    onepre = nc.dram_tensor("onepre", [NGROUPS, CHUNK], F32R,
                            kind="ExternalInput")
    outs = nc.dram_tensor("outs", [128, 1], F32, kind="ExternalOutput")

    ops = _ring_ops()
    n_aops = sum(1 for e, _, _ in ops if e == "A")
    n_vops = len(ops) - n_aops
    nred = max(n_aops, n_vops) + 1
    consumer_of_bank = {}
    for i, (e, b0, ln) in enumerate(ops):
        pass  # per-engine indices assigned below

    # Precompute per-op engine-local index and bank->op map.
    op_recs = []
    ia = iv = 0
    for e, b0, ln in ops:
        idx = ia if e == "A" else iv
        op_recs.append((e, b0, ln, idx))
        for b in range(b0, b0 + ln):
            consumer_of_bank[b] = (e, idx)
        if e == "A":
            ia += 1
        else:
            iv += 1

    with tile.TileContext(nc) as tc:
        with (
            tc.tile_pool(name="singles", bufs=1) as singles,
            tc.tile_pool(name="psall", bufs=1, space="PSUM") as psall,
        ):
            lw_t = singles.tile([128, 128 * ROW_SLOTS], F32R)
            lwv = lw_t.rearrange("(g r) c -> g r c", r=32)

            # Accumulator columns: ACT in red[:, 0, i, 0], DVE in
            # red[:, 1, i, 0] -- 16B-strided and engine-separated so writes
            # and the carriers'/observers' reads never share a dependency-
            # tracking granule (false sharing costs an extra sync wait).
            red = singles.tile([128, 2, nred, 4], F32)
            tiny_a = singles.tile([128, 2], F32)   # ACT-only scratch
            tiny_v = singles.tile([128, 8], F32)   # DVE-only scratch (padded)
            # Consumers dump relu output here (read nowhere), ping-pong so
            # the WAW lag matches the carrier-covered tick (op idx-2).
            junk_a = singles.tile([128, 2, 3 * NT + 64], F32)
            junk_v = singles.tile([128, 2, 3 * NT + 64], F32)

            rhs_all = singles.tile([128, NFILLS, CHUNK], F32R)
            rall = rhs_all.rearrange("(g r) s c -> g r (s c)", r=32)
            # PSUM as a flat 8-bank ring.
            ps = psall.tile([128, 8 * NT], F32)

            # Fill 0's data first so compute can start ASAP.
            fill_dmas = []
            rg0 = rhs_all[:, 0, :].rearrange("(g r) c -> g r c", r=32)
            for k in range(2):
                fill_dmas.append(
                    nc.sync.dma_start(out=rg0[:, k, :], in_=xd[0, k, :, :])
                )

            # Compact lhsT table on the SWDGE path; small first piece so
            # fills 0-3 unblock early.
            lw_dmas = [
                nc.gpsimd.dma_start(out=lwv[:, k, 0:1024],
                                    in_=lwt4[k][:, 0:1024])
                for k in range(4)
            ] + [
                nc.gpsimd.dma_start(out=lwv[:, k, 1024:],
                                    in_=lwt4[k][:, 1024:])
                for k in range(4)
            ]

            # Warm the ACT relu table while DMAs land (garbage in/out is
            # fine; this only forces the ACT_TABLE_LOAD off the hot path).
            wa = nc.scalar.activation(
                out=tiny_a[:, 0:1], in_=tiny_a[:, 1:2],
                func=mybir.ActivationFunctionType.Relu,
            )

            # Observers: standalone bf16 ldweights reading cells inside a
            # DMA-written (or ACT-written) region. The RAW dep makes the PE
            # sequencer observe that lane/engine tick with exactly one sync
            # wait and no PSUM write. The fp32r matmuls self-load their
            # weights, so PE array state is unharmed.
            last_obs = [None]

            def observe(cell_ap):
                ob = nc.tensor.ldweights(cell_ap.bitcast(BF16))
                if last_obs[0] is not None:
                    add_dep_helper(ob.ins, last_obs[0].ins,
                                   reason="observer chain")
                last_obs[0] = ob
                return ob

            pres = []
            act_hist = []
            dve_hist = []
            last_mm = [None]
            op_ptr = [0]

            def emit_consumer(e, b0, ln, idx):
                off = NT * (b0 % 8)
                fd = NT * ln
                if e == "A":
                    if idx >= 2:
                        # ACT carrier: one ACT-sem wait at op idx-2's tick
                        # (RAW on its red column) covers the consumer's junk
                        # WAW so the consumer only waits PE.
                        nc.scalar.activation(
                            out=tiny_a[:, 1:2],
                            in_=red[:, 0, idx - 2, 0:1],
                            func=mybir.ActivationFunctionType.Relu,
                        )
                    cons = nc.scalar.activation(
                        out=junk_a[:, idx % 2, 0:fd],
                        in_=ps[:, off : off + fd],
                        func=mybir.ActivationFunctionType.Relu,
                        accum_out=red[:, 0, idx, 0:1],
                    )
                    act_hist.append(cons.ins)
                else:
                    if idx >= 2:
                        nc.vector.tensor_scalar(
                            out=tiny_v[:, 4:5],
                            in0=red[:, 1, idx - 2, 0:1],
                            scalar1=0.0, scalar2=None,
                            op0=mybir.AluOpType.add,
                        )
                    cons = nc.vector.tensor_scalar(
                        out=junk_v[:, idx % 2, 0:fd],
                        in0=ps[:, off : off + fd],
                        scalar1=0.0,
                        scalar2=None,
                        op0=mybir.AluOpType.max,
                        op1=mybir.AluOpType.add,
                        accum_out=red[:, 1, idx, 0:1],
                    )
                    dve_hist.append(cons.ins)

            for f in range(NFILLS):
                rhs = rhs_all[:, f, :]
                rg = rhs.rearrange("(g r) c -> g r c", r=32)

                # Per-fill const rows: small DMAs (8KB per partition line)
                # that never hold up the ring.
                pres.append(
                    nc.sync.dma_start(
                        out=rall[:, 2, CHUNK * f : CHUNK * (f + 1)],
                        in_=ncpre[:],
                    )
                )
                pres.append(
                    nc.sync.dma_start(
                        out=rall[:, 3, CHUNK * f : CHUNK * (f + 1)],
                        in_=onepre[:],
                    )
                )

                if f > 0:
                    for k in range(2):
                        fill_dmas.append(
                            nc.sync.dma_start(out=rg[:, k, :],
                                              in_=xd[f, k, :, :])
                        )
                else:
                    # lwt piece-0 cascade (4 SWDGE lanes).
                    for k in range(1, 5):
                        observe(lw_t[0:k, 0:1])
                if f == 4:
                    # lwt piece-1 cascade before fill 4 needs those slots.
                    for k in range(1, 5):
                        observe(lw_t[0:k, 1024:1025])

                # Observer ldweights cascade (partition base 0, growing K)
                # absorbs this fill's four DMA-lane waits one at a time so
                # the fill's real matmuls carry only their consumer WAR.
                for k in range(1, 5):
                    observe(rhs_all[0:k, f, 0:1])

                for q in range(4):
                    Q = 4 * f + q

                    # PE pre-observes the newest ACT tick among the ops that
                    # consumed these banks one ring-lap ago, so each matmul
                    # carries at most a single (DVE) WAR wait.
                    act_idxs = [
                        consumer_of_bank[4 * Q + g - 8][1]
                        for g in range(NGROUPS)
                        if 4 * Q + g - 8 >= 0
                        and consumer_of_bank[4 * Q + g - 8][0] == "A"
                    ]
                    if act_idxs:
                        i_obs = max(act_idxs)
                        observe(red[0:1, 0, i_obs, 0:1])

                    s = 2 * f + (q // 2)
                    for g in range(NGROUPS):
                        bank = (4 * Q + g) % 8
                        last_mm[0] = nc.tensor.matmul(
                            ps[:, NT * bank : NT * (bank + 1)],
                            lw_t[32 * g : 32 * g + 4,
                                 128 * s : 128 * (s + 1)],
                            rhs[32 * g : 32 * g + 4,
                                NT * q : NT * (q + 1)],
                            start=True, stop=True,
                            tile_position=(32 * g, 0),
                        )

                    while (op_ptr[0] < len(op_recs)
                           and op_recs[op_ptr[0]][1] + op_recs[op_ptr[0]][2]
                           <= 4 * (Q + 1)):
                        emit_consumer(*op_recs[op_ptr[0]])
                        op_ptr[0] += 1

            assert op_ptr[0] == len(op_recs)

            outs_t = singles.tile([128, 1], F32)
            # DVE carrier observes the ACT tail so the reduce needs at most
            # one sync wait.
            cfin = nc.vector.tensor_scalar(
                out=tiny_v[:, 0:1], in0=red[:, 0, n_aops - 1, 0:1],
                scalar1=0.0, scalar2=None, op0=mybir.AluOpType.add,
            )
            add_dep_helper(cfin.ins, act_hist[-1],
                           reason="reduce observes ACT tail")
            rsum = nc.vector.reduce_sum(
                outs_t[:], red[:, :, 0 : max(n_aops, n_vops), 0:1],
                axis=mybir.AxisListType.XYZ,
            )
            # GpSimd pre-observes every SWDGE lane (the lwt DMAs) so the
            # output DMA's lane-reuse wait is already covered and it only
            # waits on the reduce.
            gp_scratch = singles.tile([1, len(lw_dmas), 4], F32)
            for i, d in enumerate(lw_dmas):
                gm = nc.gpsimd.memset(gp_scratch[0:1, i, 0:1], 0.0)
                add_dep_helper(gm.ins, d.ins, reason="observe SWDGE lane")
            odma = nc.gpsimd.dma_start(out=outs[:], in_=outs_t[:])

            # Pre-observe every proc on SP so the TileContext-exit drain
            # has nothing left to wait on.
            drain_deps = [act_hist[-1], dve_hist[-1], rsum.ins,
                          last_mm[0].ins, odma.ins, wa.ins, cfin.ins]
            drain_deps += [d.ins for d in lw_dmas]
            drain_deps += [p_.ins for p_ in pres[-4:]]
            drain_deps += [d.ins for d in fill_dmas[-8:]]
            for dins in drain_deps:
                dr = nc.sync.drain(fusable=False)
                add_dep_helper(dr.ins, dins, reason="pre-drain observe")

    _NC_CACHE = nc
    return nc


def kernel(x, W1, b1, W2, b2, W3, b3):
    global LAST_RESULT
    x = np.asarray(x, dtype=np.float32)
    W1 = np.asarray(W1, dtype=np.float32)
    b1 = np.asarray(b1, dtype=np.float32)
    W2 = np.asarray(W2, dtype=np.float32)
    b2 = np.asarray(b2, dtype=np.float32)
    W3 = np.asarray(W3, dtype=np.float32)
    b3 = np.asarray(b3, dtype=np.float32)

    x0, x1 = x[0], x[1]
    mask = x0 != 0.0

    rows_any = mask.any(axis=1)
    cols_any = mask.any(axis=0)
    ridx = np.nonzero(rows_any)[0]
    cidx = np.nonzero(cols_any)[0]
    rmin, rmax = float(ridx[0]), float(ridx[-1])
    cmin, cmax = float(cidx[0]), float(cidx[-1])

    W12 = W1.astype(np.float64) @ W2.astype(np.float64)
    b12 = b1.astype(np.float64) @ W2.astype(np.float64) + b2
    v0 = W12[0]

    nr_all = (np.arange(H, dtype=np.float64) - rmin) / (rmax - rmin)
    nc_all = (np.arange(W, dtype=np.float64) - cmin) / (cmax - cmin)

    nc2 = np.tile(nc_all.astype(np.float32), 2)
    v2f = W12[2].astype(np.float32)
    v3f = W12[3].astype(np.float32)
    v1f = W12[1].astype(np.float32)

    ncpre = np.broadcast_to(nc2[None, :], (NGROUPS, CHUNK)).copy()
    onepre = np.ones((NGROUPS, CHUNK), dtype=np.float32)

    nc_prog = _build_bass()
    in_maps = []
    for c in range(N_CORES):
        shard = x[:, c * ROWS_PER_CORE : (c + 1) * ROWS_PER_CORE, :]
        xdv = np.empty((NFILLS, 2, NGROUPS, CHUNK), dtype=np.float32)
        sh = shard.reshape(2, NFILLS, NGROUPS, CHUNK)
        xdv[:, 0, :, :] = sh[0]
        xdv[:, 1, :, :] = sh[1]

        btab = (
            b12[:, None]
            + np.outer(v0, nr_all[c * ROWS_PER_CORE : (c + 1) * ROWS_PER_CORE])
        ).astype(np.float32)
        lwt4 = np.empty((4, NGROUPS, 128 * ROW_SLOTS), dtype=np.float32)
        lwt4[0] = np.tile(v2f, ROW_SLOTS)[None, :]
        lwt4[1] = np.tile(v3f, ROW_SLOTS)[None, :]
        lwt4[2] = np.tile(v1f, ROW_SLOTS)[None, :]
        for g in range(NGROUPS):
            for s in range(ROW_SLOTS):
                f, half = divmod(s, 2)
                r_loc = 8 * f + 2 * g + half
                lwt4[3, g, 128 * s : 128 * (s + 1)] = btab[:, r_loc]
        in_maps.append({"xd": xdv, "lwt4": lwt4, "ncpre": ncpre,
                        "onepre": onepre})

    res = run_bass_kernel_spmd(
        nc_prog, in_maps, core_ids=list(range(N_CORES)), trace=TRACE
    )
    LAST_RESULT = res

    S = np.zeros(D, dtype=np.float64)
    for c in range(N_CORES):
        S += res.results[c]["outs"][:, 0].astype(np.float64)

    if not mask.all():
        zr, zc = np.nonzero(~mask)
        hz = (
            np.outer(nr_all[zr], W12[0])
            + np.outer(nc_all[zc], W12[1])
            + np.outer(x1[zr, zc].astype(np.float64), W12[3])
            + b12[None, :]
        )
        S -= np.maximum(hz, 0.0).sum(axis=0)

    wsum = float(mask.sum())
    out = (S @ W3.astype(np.float64)) / wsum + b3.astype(np.float64)
    return out.astype(np.float32)
